# revision 2
# baseline (speedup 1.0000x reference)
"""Trainium2 Bass kernel for MoE routing (nn_MoE_mod_67224828117003).

Reference computation (per full batch):
  logits = x.reshape(B, C*H*W) @ w_gate            # [B, E]
  top-4 gating -> gk (softmax over top-4 logits), gates (dense scatter)
  out[b,j]     = w_exp[e_bj].T-contracted 1x1 conv of x[b] + b_exp[e_bj]
  y            = log(sum_j gk[b,j] * exp(out[b,j]))
  returns (y, gates)

Distribution over 8 NeuronCores:
  - Gating: K-split. Core m computes partial logits for ALL 128 samples over
    its 1/8 slice of the C*H*W contraction (host pre-transposes that slice so
    the contraction lands on SBUF partitions). One 8KB ReduceScatter then gives
    each core the final logits for ITS 16 samples.
  - Everything else is data-parallel over batch: each core does top-k, gathers
    its samples' 4 expert weight matrices from HBM via register-offset dynamic
    DMAs, runs the per-(sample, expert) matmuls on PE, and fuses
    gk*exp(out + b_exp) as exp(out + (b_exp + ln gk)) using the ACT engine's
    free per-partition bias.
"""
import sys

if '/opt/trn_rl_repo' not in sys.path:
    sys.path.insert(0, '/opt/trn_rl_repo')

import numpy as np

import concourse.bass as bass
import concourse.bacc as bacc
import concourse.mybir as mybir
from concourse import tile
from concourse.bass_utils import run_bass_kernel_spmd

F32 = mybir.dt.float32
U32 = mybir.dt.uint32
I32 = mybir.dt.int32
AF = mybir.ActivationFunctionType
ALU = mybir.AluOpType

B, C, S, E = 128, 256, 1024, 16          # batch, channels, H*W, experts
NCORES = 8
BL = B // NCORES                          # 16 local samples per core
KL = (C * S) // NCORES                    # 32768 gating-contraction slice
P = 128
KT = KL // P                              # 256 gating k-tiles
GCHUNK = 16                               # k-tiles per gating DMA batch

# FFN matmul dtype. float32r runs the PE at 1 col/cycle (vs 4 for float32) at
# reduced multiply precision; gating stays float32 (expert selection needs it).
FFN_DT = F32


def build_program(ffn_dt=FFN_DT):
    nc = bacc.Bacc("TRN2", target_bir_lowering=False, debug=False,
                   num_devices=NCORES)

    xg = nc.dram_tensor("xg", [KL, B], F32, kind="ExternalInput")
    wg = nc.dram_tensor("wg", [KL, E], F32, kind="ExternalInput")
    xl = nc.dram_tensor("xl", [BL, C, S], F32, kind="ExternalInput")
    wexp = nc.dram_tensor("wexp", [E, 2, P, C], F32, kind="ExternalInput")
    bexp = nc.dram_tensor("bexp", [E, C], F32, kind="ExternalInput")
    ident = nc.dram_tensor("ident", [16, 16], F32, kind="ExternalInput")

    y = nc.dram_tensor("y", [BL, 2, P, S], F32, kind="ExternalOutput")
    gates = nc.dram_tensor("gates", [BL, E], F32, kind="ExternalOutput")

    with tile.TileContext(nc) as tc:
        with tc.tile_pool(name="small", bufs=1) as small, \
             tc.tile_pool(name="dram", bufs=1, space="DRAM") as dr:

            # ---- Phase 1: gating partial logits over this core's K-slice ----
            lg_sb = small.tile([P, E], F32)
            with tc.tile_pool(name="gload", bufs=3) as gl, \
                 tc.tile_pool(name="gps", bufs=1, space="PSUM") as gps:
                lg_ps = gps.tile([P, E], F32)
                for g in range(KT // GCHUNK):
                    r0 = g * GCHUNK * P
                    xg_t = gl.tile([P, GCHUNK * B], F32, tag="xg")
                    nc.sync.dma_start(
                        xg_t[:],
                        xg[r0:r0 + GCHUNK * P, :].rearrange(
                            "(kt p) s -> p kt s", p=P))
                    wg_t = gl.tile([P, GCHUNK * E], F32, tag="wg")
                    nc.sync.dma_start(
                        wg_t[:],
                        wg[r0:r0 + GCHUNK * P, :].rearrange(
                            "(kt p) e -> p kt e", p=P))
                    for kt in range(GCHUNK):
                        nc.tensor.matmul(
                            lg_ps[:],
                            lhsT=xg_t[:, kt * B:(kt + 1) * B],
                            rhs=wg_t[:, kt * E:(kt + 1) * E],
                            start=(g == 0 and kt == 0),
                            stop=(g == KT // GCHUNK - 1 and kt == GCHUNK - 1))
                nc.vector.tensor_copy(lg_sb[:], lg_ps[:])

            # ---- Phase 2: ReduceScatter -> this core's final logits ----
            cc_in = dr.tile([P, E], F32)
            cc_out = dr.tile([BL, E], F32)
            nc.sync.dma_start(cc_in[:], lg_sb[:])
            nc.gpsimd.collective_compute(
                "ReduceScatter", ALU.add,
                replica_groups=[list(range(NCORES))],
                ins=[cc_in.opt()], outs=[cc_out.opt()])
            lg_loc = small.tile([BL, E], F32)
            nc.sync.dma_start(lg_loc[:], cc_out[:])

            # ---- Phase 3: top-4, softmax, gates, bias table ----
            m8 = small.tile([BL, 8], F32)
            i8 = small.tile([BL, 8], U32)
            nc.vector.max(out=m8[:], in_=lg_loc[:])
            nc.vector.max_index(out=i8[:], in_max=m8[:], in_values=lg_loc[:])

            d = small.tile([BL, 4], F32)
            nc.vector.tensor_scalar(d[:], m8[:, 0:4], m8[:, 0:1], None,
                                    op0=ALU.subtract)
            ed = small.tile([BL, 4], F32)
            nc.scalar.activation(ed[:], d[:], AF.Exp)
            ssum = small.tile([BL, 1], F32)
            nc.vector.tensor_reduce(ssum[:], ed[:], axis=mybir.AxisListType.X,
                                    op=ALU.add)
            rsum = small.tile([BL, 1], F32)
            nc.vector.reciprocal(rsum[:], ssum[:])
            gk = small.tile([BL, 4], F32)
            nc.vector.tensor_scalar_mul(gk[:], ed[:], rsum[:])
            lnsum = small.tile([BL, 1], F32)
            nc.scalar.activation(lnsum[:], ssum[:], AF.Ln)
            lngk = small.tile([BL, 4], F32)
            nc.vector.tensor_scalar(lngk[:], d[:], lnsum[:], None,
                                    op0=ALU.subtract)

            ident_sb = small.tile([16, 16], F32)
            nc.sync.dma_start(ident_sb[:], ident[:])
            gates_sb = small.tile([BL, E], F32)
            nc.vector.memset(gates_sb[:], 0.0)
            gt_sb = small.tile([16, 4 * BL], F32)
            bias_all = small.tile([P, 2 * 4 * BL], F32)
            with tc.tile_pool(name="tps", bufs=1, space="PSUM") as tps:
                for r in range(4):
                    eq = small.tile([BL, E], F32, tag=f"eq{r}")
                    nc.vector.tensor_scalar(eq[:], lg_loc[:], m8[:, r:r + 1],
                                            None, op0=ALU.is_equal)
                    nc.vector.scalar_tensor_tensor(
                        out=gates_sb[:], in0=eq[:], scalar=gk[:, r:r + 1],
                        in1=gates_sb[:], op0=ALU.mult, op1=ALU.add)
                    tp = tps.tile([16, 16], F32, tag=f"tp{r}")
                    nc.tensor.transpose(tp[:], eq[:], ident_sb[:])
                    nc.vector.tensor_copy(gt_sb[:, r * BL:(r + 1) * BL], tp[:])
                nc.sync.dma_start(gates[:], gates_sb[:])

                # bias_all[:, mc*64 + j*16 + s] = b_exp[e_sj, mc*128+p] + ln gk[s,j]
                lngk_d = dr.tile([1, 4 * BL], F32)
                nc.sync.dma_start(
                    lngk_d[0:1, :].rearrange("o (j s) -> o s j", s=BL),
                    lngk[:])
                lngk_bc = small.tile([P, 4 * BL], F32)
                nc.sync.dma_start(lngk_bc[:],
                                  lngk_d[0:1, :].to_broadcast([P, 4 * BL]))
                for mc in range(2):
                    bx = small.tile([16, P], F32, tag=f"bx{mc}")
                    nc.sync.dma_start(bx[:], bexp[:, mc * P:(mc + 1) * P])
                    bps = tps.tile([P, 4 * BL], F32, tag=f"bps{mc}")
                    nc.tensor.matmul(bps[:], lhsT=bx[:], rhs=gt_sb[:],
                                     start=True, stop=True)
                    nc.vector.tensor_add(
                        bias_all[:, mc * 4 * BL:(mc + 1) * 4 * BL],
                        bps[:], lngk_bc[:])

            # ---- Phase 4: per-sample expert FFN + combine ----
            with tc.tile_pool(name="ffn", bufs=2) as fp, \
                 tc.tile_pool(name="fwt", bufs=4) as fw, \
                 tc.tile_pool(name="fps", bufs=2, space="PSUM") as fps:
                for s in range(BL):
                    xt = []
                    for kc in range(2):
                        t = fp.tile([P, S], F32, tag=f"xt{kc}")
                        if ffn_dt != F32:
                            t = t.bitcast(ffn_dt)
                        nc.sync.dma_start(t[:], xl[s, kc * P:(kc + 1) * P, :])
                        xt.append(t)
                    acc = fp.tile([P, 2 * S], F32, tag="acc")
                    for j in range(4):
                        ev = nc.values_load(
                            i8[s:s + 1, j:j + 1].bitcast(I32),
                            engines=(mybir.EngineType.SP,),
                            min_val=0, max_val=E - 1,
                            skip_runtime_bounds_check=True)
                        wt = fw.tile([P, 2 * C], F32, tag="wt")
                        if ffn_dt != F32:
                            wt = wt.bitcast(ffn_dt)
                        nc.sync.dma_start(
                            wt[:],
                            wexp[bass.ds(ev, 1), :, :, :].rearrange(
                                "e kc p o -> e p kc o"))
                        ps0 = fps.tile([P, S], F32, tag="ps0")
                        ps1 = fps.tile([P, S], F32, tag="ps1")
                        ps = [ps0, ps1]
                        for mc in range(2):
                            for kc in range(2):
                                lhsT = wt[:, kc * C + mc * P:kc * C + mc * P + P]
                                for n_ in range(2):
                                    nc.tensor.matmul(
                                        ps[mc][:, n_ * 512:(n_ + 1) * 512],
                                        lhsT=lhsT,
                                        rhs=xt[kc][:, n_ * 512:(n_ + 1) * 512],
                                        start=(kc == 0), stop=(kc == 1))
                        for mc in range(2):
                            bias_ap = bias_all[:, mc * 4 * BL + j * BL + s:
                                               mc * 4 * BL + j * BL + s + 1]
                            if j == 0:
                                nc.scalar.activation(
                                    acc[:, mc * S:(mc + 1) * S], ps[mc][:],
                                    AF.Exp, bias=bias_ap)
                            else:
                                et = fp.tile([P, S], F32, tag="etmp")
                                nc.scalar.activation(et[:], ps[mc][:],
                                                     AF.Exp, bias=bias_ap)
                                nc.vector.tensor_add(
                                    acc[:, mc * S:(mc + 1) * S],
                                    acc[:, mc * S:(mc + 1) * S], et[:])
                    yt = fp.tile([P, 2 * S], F32, tag="yt")
                    nc.scalar.activation(yt[:], acc[:], AF.Ln)
                    nc.sync.dma_start(
                        y[s].rearrange("mc p hw -> p mc hw"), yt[:])

    nc.compile()
    return nc


def shard_inputs(x, w_gate, w_exp, b_exp):
    """Build the 8 per-core input maps from the full-problem arrays."""
    x = np.ascontiguousarray(x, dtype=np.float32)
    w_gate = np.ascontiguousarray(w_gate, dtype=np.float32)
    w_exp = np.ascontiguousarray(w_exp, dtype=np.float32)
    b_exp = np.ascontiguousarray(b_exp, dtype=np.float32)

    xf = x.reshape(B, C * S)
    wexp_r = w_exp.reshape(E, 2, P, C)
    ident = np.eye(16, dtype=np.float32)

    in_maps = []
    for m in range(NCORES):
        ks = slice(m * KL, (m + 1) * KL)
        in_maps.append({
            "xg": np.ascontiguousarray(xf[:, ks].T),
            "wg": w_gate[ks],
            "xl": x[m * BL:(m + 1) * BL].reshape(BL, C, S),
            "wexp": wexp_r,
            "bexp": b_exp,
            "ident": ident,
        })
    return in_maps


_NC_CACHE = {}


def get_program(ffn_dt=FFN_DT):
    if ffn_dt not in _NC_CACHE:
        _NC_CACHE[ffn_dt] = build_program(ffn_dt)
    return _NC_CACHE[ffn_dt]


def run_sharded(inputs, trace=False, ffn_dt=FFN_DT):
    nc = get_program(ffn_dt)
    in_maps = shard_inputs(inputs["x"], inputs["w_gate"], inputs["w_exp"],
                           inputs["b_exp"])
    res = run_bass_kernel_spmd(nc, in_maps, list(range(NCORES)), trace=trace)
    y = np.concatenate([r["y"].reshape(BL, C, 32, 32) for r in res.results])
    gates = np.concatenate([r["gates"] for r in res.results])
    return (y, gates), res


def kernel(x, w_gate, w_exp, b_exp, k):
    assert int(k) == 4, f"kernel hardcodes top-4 gating, got k={k}"
    (y, gates), _ = run_sharded(
        {"x": np.asarray(x), "w_gate": np.asarray(w_gate),
         "w_exp": np.asarray(w_exp), "b_exp": np.asarray(b_exp)})
    return y, gates


# revision 5
# speedup vs baseline: 23895.8484x; 23895.8484x over previous
"""Trainium2 Bass kernel for MoE routing (nn_MoE_mod_67224828117003).

Reference computation (per full batch):
  logits = x.reshape(B, C*H*W) @ w_gate            # [B, E]
  top-4 gating -> gk (softmax over top-4 logits), gates (dense scatter)
  out[b,j]     = w_exp[e_bj].T-contracted 1x1 conv of x[b] + b_exp[e_bj]
  y            = log(sum_j gk[b,j] * exp(out[b,j]))
  returns (y, gates)

Distribution over 8 NeuronCores:
  - Gating: K-split. Core m computes partial logits for ALL 128 samples over
    its 1/8 slice of the C*H*W contraction (host pre-transposes that slice so
    the contraction lands on SBUF partitions). One 8KB ReduceScatter then gives
    each core the final logits for ITS 16 samples.
  - Everything else is data-parallel over batch: each core does top-k, gathers
    its samples' 4 expert weight matrices from HBM via register-offset dynamic
    DMAs, runs the per-(sample, expert) matmuls on PE, and fuses
    gk*exp(out + b_exp) as exp(out + (b_exp + ln gk)) using the ACT engine's
    free per-partition bias.
"""
import sys

if '/opt/trn_rl_repo' not in sys.path:
    sys.path.insert(0, '/opt/trn_rl_repo')

import numpy as np

import concourse.bass as bass
import concourse.bacc as bacc
import concourse.mybir as mybir
from concourse import tile
from concourse.bass_utils import run_bass_kernel_spmd

F32 = mybir.dt.float32
U32 = mybir.dt.uint32
I32 = mybir.dt.int32
AF = mybir.ActivationFunctionType
ALU = mybir.AluOpType

B, C, S, E = 128, 256, 1024, 16          # batch, channels, H*W, experts
NCORES = 8
BL = B // NCORES                          # 16 local samples per core
KL = (C * S) // NCORES                    # 32768 gating-contraction slice
P = 128
KT = KL // P                              # 256 gating k-tiles
GCHUNK = 16                               # k-tiles per gating DMA batch

# FFN matmul dtype. float32r runs the PE at 1 col/cycle (vs 4 for float32) at
# reduced multiply precision; gating stays float32 (expert selection needs it).
FFN_DT = F32


def build_program(ffn_dt=FFN_DT):
    nc = bacc.Bacc("TRN2", target_bir_lowering=False, debug=False,
                   num_devices=NCORES)

    xg = nc.dram_tensor("xg", [KL, B], F32, kind="ExternalInput")
    wg = nc.dram_tensor("wg", [KL, E], F32, kind="ExternalInput")
    xl = nc.dram_tensor("xl", [BL, C, S], F32, kind="ExternalInput")
    wexp = nc.dram_tensor("wexp", [E, 2, P, C], F32, kind="ExternalInput")
    bexp = nc.dram_tensor("bexp", [E, C], F32, kind="ExternalInput")
    ident = nc.dram_tensor("ident", [16, 16], F32, kind="ExternalInput")

    y = nc.dram_tensor("y", [BL, 2, P, S], F32, kind="ExternalOutput")
    gates = nc.dram_tensor("gates", [BL, E], F32, kind="ExternalOutput")

    with tile.TileContext(nc) as tc:
        with tc.tile_pool(name="small", bufs=1) as small, \
             tc.tile_pool(name="dram", bufs=1, space="DRAM") as dr:

            # ---- Phase 1: gating partial logits over this core's K-slice ----
            lg_sb = small.tile([P, E], F32)
            with tc.tile_pool(name="gload", bufs=3) as gl, \
                 tc.tile_pool(name="gps", bufs=1, space="PSUM") as gps:
                lg_ps = gps.tile([P, E], F32)
                for g in range(KT // GCHUNK):
                    r0 = g * GCHUNK * P
                    xg_t = gl.tile([P, GCHUNK * B], F32, tag="xg")
                    nc.sync.dma_start(
                        xg_t[:],
                        xg[r0:r0 + GCHUNK * P, :].rearrange(
                            "(kt p) s -> p kt s", p=P))
                    wg_t = gl.tile([P, GCHUNK * E], F32, tag="wg")
                    nc.sync.dma_start(
                        wg_t[:],
                        wg[r0:r0 + GCHUNK * P, :].rearrange(
                            "(kt p) e -> p kt e", p=P))
                    for kt in range(GCHUNK):
                        nc.tensor.matmul(
                            lg_ps[:],
                            lhsT=xg_t[:, kt * B:(kt + 1) * B],
                            rhs=wg_t[:, kt * E:(kt + 1) * E],
                            start=(g == 0 and kt == 0),
                            stop=(g == KT // GCHUNK - 1 and kt == GCHUNK - 1))
                nc.vector.tensor_copy(lg_sb[:], lg_ps[:])

            # ---- Phase 2: ReduceScatter -> this core's final logits ----
            cc_in = dr.tile([P, E], F32)
            cc_out = dr.tile([BL, E], F32)
            nc.sync.dma_start(cc_in[:], lg_sb[:])
            nc.gpsimd.collective_compute(
                "ReduceScatter", ALU.add,
                replica_groups=[list(range(NCORES))],
                ins=[cc_in.opt()], outs=[cc_out.opt()])
            lg_loc = small.tile([BL, E], F32)
            nc.sync.dma_start(lg_loc[:], cc_out[:])

            # ---- Phase 3: top-4, softmax, gates, bias table ----
            m8 = small.tile([BL, 8], F32)
            i8 = small.tile([BL, 8], U32)
            nc.vector.max(out=m8[:], in_=lg_loc[:])
            nc.vector.max_index(out=i8[:], in_max=m8[:], in_values=lg_loc[:])

            d = small.tile([BL, 4], F32)
            nc.vector.tensor_scalar(d[:], m8[:, 0:4], m8[:, 0:1], None,
                                    op0=ALU.subtract)
            ed = small.tile([BL, 4], F32)
            nc.scalar.activation(ed[:], d[:], AF.Exp)
            ssum = small.tile([BL, 1], F32)
            nc.vector.tensor_reduce(ssum[:], ed[:], axis=mybir.AxisListType.X,
                                    op=ALU.add)
            rsum = small.tile([BL, 1], F32)
            nc.vector.reciprocal(rsum[:], ssum[:])
            gk = small.tile([BL, 4], F32)
            nc.vector.tensor_scalar_mul(gk[:], ed[:], rsum[:])
            lnsum = small.tile([BL, 1], F32)
            nc.scalar.activation(lnsum[:], ssum[:], AF.Ln)
            lngk = small.tile([BL, 4], F32)
            nc.vector.tensor_scalar(lngk[:], d[:], lnsum[:], None,
                                    op0=ALU.subtract)

            ident_sb = small.tile([16, 16], F32)
            nc.sync.dma_start(ident_sb[:], ident[:])
            gates_sb = small.tile([BL, E], F32)
            nc.vector.memset(gates_sb[:], 0.0)
            gt_sb = small.tile([16, 4 * BL], F32)
            bias_all = small.tile([P, 2 * 4 * BL], F32)
            with tc.tile_pool(name="tps", bufs=1, space="PSUM") as tps:
                for r in range(4):
                    eq = small.tile([BL, E], F32, tag=f"eq{r}")
                    nc.vector.tensor_scalar(eq[:], lg_loc[:], m8[:, r:r + 1],
                                            None, op0=ALU.is_equal)
                    nc.vector.scalar_tensor_tensor(
                        out=gates_sb[:], in0=eq[:], scalar=gk[:, r:r + 1],
                        in1=gates_sb[:], op0=ALU.mult, op1=ALU.add)
                    tp = tps.tile([16, 16], F32, tag=f"tp{r}")
                    nc.tensor.transpose(tp[:], eq[:], ident_sb[:])
                    nc.vector.tensor_copy(gt_sb[:, r * BL:(r + 1) * BL], tp[:])
                nc.sync.dma_start(gates[:], gates_sb[:])

                # bias_all[:, mc*64 + j*16 + s] = b_exp[e_sj, mc*128+p] + ln gk[s,j]
                lngk_d = dr.tile([1, 4 * BL], F32)
                nc.sync.dma_start(
                    lngk_d[0:1, :].rearrange("o (j s) -> o s j", s=BL),
                    lngk[:])
                lngk_bc = small.tile([P, 4 * BL], F32)
                nc.sync.dma_start(lngk_bc[:],
                                  lngk_d[0:1, :].to_broadcast([P, 4 * BL]))
                for mc in range(2):
                    bx = small.tile([16, P], F32, tag=f"bx{mc}")
                    nc.sync.dma_start(bx[:], bexp[:, mc * P:(mc + 1) * P])
                    bps = tps.tile([P, 4 * BL], F32, tag=f"bps{mc}")
                    nc.tensor.matmul(bps[:], lhsT=bx[:], rhs=gt_sb[:],
                                     start=True, stop=True)
                    nc.vector.tensor_add(
                        bias_all[:, mc * 4 * BL:(mc + 1) * 4 * BL],
                        bps[:], lngk_bc[:])

            # ---- Phase 4: per-sample expert FFN + combine ----
            with tc.tile_pool(name="ffn", bufs=2) as fp, \
                 tc.tile_pool(name="fwt", bufs=4) as fw, \
                 tc.tile_pool(name="fps", bufs=2, space="PSUM") as fps:
                for s in range(BL):
                    xt = []
                    for kc in range(2):
                        t = fp.tile([P, S], F32, tag=f"xt{kc}")
                        nc.sync.dma_start(t[:], xl[s, kc * P:(kc + 1) * P, :])
                        xt.append(t)
                    acc = fp.tile([P, 2 * S], F32, tag="acc")
                    for j in range(4):
                        ev = nc.values_load(
                            i8[s:s + 1, j:j + 1].bitcast(I32),
                            engines=(mybir.EngineType.SP,),
                            min_val=0, max_val=E - 1,
                            skip_runtime_bounds_check=True)
                        wt = fw.tile([P, 2 * C], F32, tag="wt")
                        nc.sync.dma_start(
                            wt[:],
                            wexp[bass.ds(ev, 1), :, :, :].rearrange(
                                "e kc p o -> e p kc o"))
                        ps0 = fps.tile([P, S], F32, tag="ps0")
                        ps1 = fps.tile([P, S], F32, tag="ps1")
                        ps = [ps0, ps1]
                        for mc in range(2):
                            for kc in range(2):
                                lhsT = wt[:, kc * C + mc * P:kc * C + mc * P + P]
                                rhs_t = xt[kc]
                                if ffn_dt != F32:
                                    lhsT = lhsT.bitcast(ffn_dt)
                                    rhs_t = rhs_t.bitcast(ffn_dt)
                                for n_ in range(2):
                                    nc.tensor.matmul(
                                        ps[mc][:, n_ * 512:(n_ + 1) * 512],
                                        lhsT=lhsT,
                                        rhs=rhs_t[:, n_ * 512:(n_ + 1) * 512],
                                        start=(kc == 0), stop=(kc == 1))
                        for mc in range(2):
                            bias_ap = bias_all[:, mc * 4 * BL + j * BL + s:
                                               mc * 4 * BL + j * BL + s + 1]
                            if j == 0:
                                nc.scalar.activation(
                                    acc[:, mc * S:(mc + 1) * S], ps[mc][:],
                                    AF.Exp, bias=bias_ap)
                            else:
                                et = fp.tile([P, S], F32, tag="etmp")
                                nc.scalar.activation(et[:], ps[mc][:],
                                                     AF.Exp, bias=bias_ap)
                                nc.vector.tensor_add(
                                    acc[:, mc * S:(mc + 1) * S],
                                    acc[:, mc * S:(mc + 1) * S], et[:])
                    yt = fp.tile([P, 2 * S], F32, tag="yt")
                    nc.scalar.activation(yt[:], acc[:], AF.Ln)
                    nc.sync.dma_start(
                        y[s].rearrange("mc p hw -> p mc hw"), yt[:])

    nc.compile()
    return nc


def shard_inputs(x, w_gate, w_exp, b_exp):
    """Build the 8 per-core input maps from the full-problem arrays."""
    x = np.ascontiguousarray(x, dtype=np.float32)
    w_gate = np.ascontiguousarray(w_gate, dtype=np.float32)
    w_exp = np.ascontiguousarray(w_exp, dtype=np.float32)
    b_exp = np.ascontiguousarray(b_exp, dtype=np.float32)

    xf = x.reshape(B, C * S)
    wexp_r = w_exp.reshape(E, 2, P, C)
    ident = np.eye(16, dtype=np.float32)

    in_maps = []
    for m in range(NCORES):
        ks = slice(m * KL, (m + 1) * KL)
        in_maps.append({
            "xg": np.ascontiguousarray(xf[:, ks].T),
            "wg": w_gate[ks],
            "xl": x[m * BL:(m + 1) * BL].reshape(BL, C, S),
            "wexp": wexp_r,
            "bexp": b_exp,
            "ident": ident,
        })
    return in_maps


_NC_CACHE = {}


def get_program(ffn_dt=FFN_DT):
    if ffn_dt not in _NC_CACHE:
        _NC_CACHE[ffn_dt] = build_program(ffn_dt)
    return _NC_CACHE[ffn_dt]


def run_sharded(inputs, trace=False, ffn_dt=FFN_DT):
    nc = get_program(ffn_dt)
    in_maps = shard_inputs(inputs["x"], inputs["w_gate"], inputs["w_exp"],
                           inputs["b_exp"])
    res = run_bass_kernel_spmd(nc, in_maps, list(range(NCORES)), trace=trace)
    y = np.concatenate([r["y"].reshape(BL, C, 32, 32) for r in res.results])
    gates = np.concatenate([r["gates"] for r in res.results])
    return (y, gates), res


def kernel(x, w_gate, w_exp, b_exp, k):
    assert int(k) == 4, f"kernel hardcodes top-4 gating, got k={k}"
    (y, gates), _ = run_sharded(
        {"x": np.asarray(x), "w_gate": np.asarray(w_gate),
         "w_exp": np.asarray(w_exp), "b_exp": np.asarray(b_exp)})
    return y, gates


# revision 9
# speedup vs baseline: 33995.0009x; 1.4226x over previous
"""Trainium2 Bass kernel for MoE routing (nn_MoE_mod_67224828117003).

Reference computation (per full batch):
  logits = x.reshape(B, C*H*W) @ w_gate            # [B, E]
  top-4 gating -> gk (softmax over top-4 logits), gates (dense scatter)
  out[b,j]     = w_exp[e_bj].T-contracted 1x1 conv of x[b] + b_exp[e_bj]
  y            = log(sum_j gk[b,j] * exp(out[b,j]))
  returns (y, gates)

Distribution over 8 NeuronCores:
  - Gating: K-split. Core m computes partial logits for ALL 128 samples over
    its 1/8 slice of the C*H*W contraction (host pre-transposes that slice so
    the contraction lands on SBUF partitions). One 8KB ReduceScatter then gives
    each core the final logits for ITS 16 samples.
  - Everything else is data-parallel over batch: each core does top-k, gathers
    its samples' 4 expert weight matrices from HBM via register-offset dynamic
    DMAs, runs the per-(sample, expert) matmuls on PE, and fuses
    gk*exp(out + b_exp) as exp(out + (b_exp + ln gk)) using the ACT engine's
    free per-partition bias.
"""
import sys

if '/opt/trn_rl_repo' not in sys.path:
    sys.path.insert(0, '/opt/trn_rl_repo')

import numpy as np

import concourse.bass as bass
import concourse.bacc as bacc
import concourse.mybir as mybir
from concourse import tile
from concourse.bass_utils import run_bass_kernel_spmd

F32 = mybir.dt.float32
U32 = mybir.dt.uint32
I32 = mybir.dt.int32
AF = mybir.ActivationFunctionType
ALU = mybir.AluOpType

B, C, S, E = 128, 256, 1024, 16          # batch, channels, H*W, experts
NCORES = 8
BL = B // NCORES                          # 16 local samples per core
KL = (C * S) // NCORES                    # 32768 gating-contraction slice
P = 128
KT = KL // P                              # 256 gating k-tiles
GCHUNK = 16                               # k-tiles per gating DMA batch

# FFN matmul dtype. float32r runs the PE at 1 col/cycle (vs 4 for float32) at
# reduced multiply precision; gating stays float32 (expert selection needs it).
FFN_DT = F32


def build_program(ffn_dt=FFN_DT):
    nc = bacc.Bacc("TRN2", target_bir_lowering=False, debug=False,
                   num_devices=NCORES)

    xg = nc.dram_tensor("xg", [KL, B], F32, kind="ExternalInput")
    wg = nc.dram_tensor("wg", [KL, E], F32, kind="ExternalInput")
    xl = nc.dram_tensor("xl", [BL, C, S], ffn_dt, kind="ExternalInput")
    wexp = nc.dram_tensor("wexp", [E, 2, P, C], ffn_dt, kind="ExternalInput")
    bexp = nc.dram_tensor("bexp", [E, C], F32, kind="ExternalInput")
    ident = nc.dram_tensor("ident", [16, 16], F32, kind="ExternalInput")

    y = nc.dram_tensor("y", [BL, 2, P, S], F32, kind="ExternalOutput")
    gates = nc.dram_tensor("gates", [BL, E], F32, kind="ExternalOutput")

    with tile.TileContext(nc) as tc:
        with tc.tile_pool(name="small", bufs=1) as small, \
             tc.tile_pool(name="dram", bufs=1, space="DRAM") as dr:

            # ---- Phase 1: gating partial logits over this core's K-slice ----
            lg_sb = small.tile([P, E], F32)
            with tc.tile_pool(name="gload", bufs=3) as gl, \
                 tc.tile_pool(name="gps", bufs=1, space="PSUM") as gps:
                lg_ps = gps.tile([P, E], F32)
                for g in range(KT // GCHUNK):
                    r0 = g * GCHUNK * P
                    xg_t = gl.tile([P, GCHUNK * B], F32, tag="xg")
                    nc.sync.dma_start(
                        xg_t[:],
                        xg[r0:r0 + GCHUNK * P, :].rearrange(
                            "(kt p) s -> p kt s", p=P))
                    wg_t = gl.tile([P, GCHUNK * E], F32, tag="wg")
                    nc.sync.dma_start(
                        wg_t[:],
                        wg[r0:r0 + GCHUNK * P, :].rearrange(
                            "(kt p) e -> p kt e", p=P))
                    for kt in range(GCHUNK):
                        nc.tensor.matmul(
                            lg_ps[:],
                            lhsT=xg_t[:, kt * B:(kt + 1) * B],
                            rhs=wg_t[:, kt * E:(kt + 1) * E],
                            start=(g == 0 and kt == 0),
                            stop=(g == KT // GCHUNK - 1 and kt == GCHUNK - 1))
                nc.vector.tensor_copy(lg_sb[:], lg_ps[:])

            # ---- Phase 2: ReduceScatter -> this core's final logits ----
            cc_in = dr.tile([P, E], F32)
            cc_out = dr.tile([BL, E], F32)
            nc.sync.dma_start(cc_in[:], lg_sb[:])
            nc.gpsimd.collective_compute(
                "ReduceScatter", ALU.add,
                replica_groups=[list(range(NCORES))],
                ins=[cc_in.opt()], outs=[cc_out.opt()])
            lg_loc = small.tile([BL, E], F32)
            nc.sync.dma_start(lg_loc[:], cc_out[:])

            # ---- Phase 3: top-4, softmax, gates, bias table ----
            m8 = small.tile([BL, 8], F32)
            i8 = small.tile([BL, 8], U32)
            nc.vector.max(out=m8[:], in_=lg_loc[:])
            nc.vector.max_index(out=i8[:], in_max=m8[:], in_values=lg_loc[:])

            d = small.tile([BL, 4], F32)
            nc.vector.tensor_scalar(d[:], m8[:, 0:4], m8[:, 0:1], None,
                                    op0=ALU.subtract)
            ed = small.tile([BL, 4], F32)
            nc.scalar.activation(ed[:], d[:], AF.Exp)
            ssum = small.tile([BL, 1], F32)
            nc.vector.tensor_reduce(ssum[:], ed[:], axis=mybir.AxisListType.X,
                                    op=ALU.add)
            rsum = small.tile([BL, 1], F32)
            nc.vector.reciprocal(rsum[:], ssum[:])
            gk = small.tile([BL, 4], F32)
            nc.vector.tensor_scalar_mul(gk[:], ed[:], rsum[:])
            lnsum = small.tile([BL, 1], F32)
            nc.scalar.activation(lnsum[:], ssum[:], AF.Ln)
            lngk = small.tile([BL, 4], F32)
            nc.vector.tensor_scalar(lngk[:], d[:], lnsum[:], None,
                                    op0=ALU.subtract)

            ident_sb = small.tile([16, 16], F32)
            nc.sync.dma_start(ident_sb[:], ident[:])
            gates_sb = small.tile([BL, E], F32)
            nc.vector.memset(gates_sb[:], 0.0)
            gt_sb = small.tile([16, 4 * BL], F32)
            bias_all = small.tile([P, 2 * 4 * BL], F32)
            with tc.tile_pool(name="tps", bufs=1, space="PSUM") as tps:
                for r in range(4):
                    eq = small.tile([BL, E], F32, tag=f"eq{r}")
                    nc.vector.tensor_scalar(eq[:], lg_loc[:], m8[:, r:r + 1],
                                            None, op0=ALU.is_equal)
                    nc.vector.scalar_tensor_tensor(
                        out=gates_sb[:], in0=eq[:], scalar=gk[:, r:r + 1],
                        in1=gates_sb[:], op0=ALU.mult, op1=ALU.add)
                    tp = tps.tile([16, 16], F32, tag=f"tp{r}")
                    nc.tensor.transpose(tp[:], eq[:], ident_sb[:])
                    nc.vector.tensor_copy(gt_sb[:, r * BL:(r + 1) * BL], tp[:])
                nc.sync.dma_start(gates[:], gates_sb[:])

                # bias_all[:, mc*64 + j*16 + s] = b_exp[e_sj, mc*128+p] + ln gk[s,j]
                lngk_d = dr.tile([1, 4 * BL], F32)
                nc.sync.dma_start(
                    lngk_d[0:1, :].rearrange("o (j s) -> o s j", s=BL),
                    lngk[:])
                lngk_bc = small.tile([P, 4 * BL], F32)
                nc.sync.dma_start(lngk_bc[:],
                                  lngk_d[0:1, :].to_broadcast([P, 4 * BL]))
                for mc in range(2):
                    bx = small.tile([16, P], F32, tag=f"bx{mc}")
                    nc.sync.dma_start(bx[:], bexp[:, mc * P:(mc + 1) * P])
                    bps = tps.tile([P, 4 * BL], F32, tag=f"bps{mc}")
                    nc.tensor.matmul(bps[:], lhsT=bx[:], rhs=gt_sb[:],
                                     start=True, stop=True)
                    nc.vector.tensor_add(
                        bias_all[:, mc * 4 * BL:(mc + 1) * 4 * BL],
                        bps[:], lngk_bc[:])

            # ---- Phase 4: per-sample expert FFN + combine ----
            with tc.tile_pool(name="ffn", bufs=2) as fp, \
                 tc.tile_pool(name="fwt", bufs=4) as fw, \
                 tc.tile_pool(name="fps", bufs=2, space="PSUM") as fps:
                for s in range(BL):
                    xt = []
                    for kc in range(2):
                        t = fp.tile([P, S], ffn_dt, tag=f"xt{kc}")
                        nc.sync.dma_start(t[:], xl[s, kc * P:(kc + 1) * P, :])
                        xt.append(t)
                    acc = fp.tile([P, 2 * S], F32, tag="acc")
                    for j in range(4):
                        ev = nc.values_load(
                            i8[s:s + 1, j:j + 1].bitcast(I32),
                            engines=(mybir.EngineType.SP,),
                            min_val=0, max_val=E - 1,
                            skip_runtime_bounds_check=True)
                        wt = fw.tile([P, 2 * C], ffn_dt, tag="wt")
                        nc.sync.dma_start(
                            wt[:],
                            wexp[bass.ds(ev, 1), :, :, :].rearrange(
                                "e kc p o -> e p kc o"))
                        ps0 = fps.tile([P, S], F32, tag="ps0")
                        ps1 = fps.tile([P, S], F32, tag="ps1")
                        ps = [ps0, ps1]
                        for mc in range(2):
                            for kc in range(2):
                                lhsT = wt[:, kc * C + mc * P:kc * C + mc * P + P]
                                for n_ in range(2):
                                    nc.tensor.matmul(
                                        ps[mc][:, n_ * 512:(n_ + 1) * 512],
                                        lhsT=lhsT,
                                        rhs=xt[kc][:, n_ * 512:(n_ + 1) * 512],
                                        start=(kc == 0), stop=(kc == 1))
                        for mc in range(2):
                            bias_ap = bias_all[:, mc * 4 * BL + j * BL + s:
                                               mc * 4 * BL + j * BL + s + 1]
                            if j == 0:
                                nc.scalar.activation(
                                    acc[:, mc * S:(mc + 1) * S], ps[mc][:],
                                    AF.Exp, bias=bias_ap)
                            else:
                                et = fp.tile([P, S], F32, tag="etmp")
                                nc.scalar.activation(et[:], ps[mc][:],
                                                     AF.Exp, bias=bias_ap)
                                nc.vector.tensor_add(
                                    acc[:, mc * S:(mc + 1) * S],
                                    acc[:, mc * S:(mc + 1) * S], et[:])
                    yt = fp.tile([P, 2 * S], F32, tag="yt")
                    nc.scalar.activation(yt[:], acc[:], AF.Ln)
                    nc.sync.dma_start(
                        y[s].rearrange("mc p hw -> p mc hw"), yt[:])

    nc.compile()
    return nc


def shard_inputs(x, w_gate, w_exp, b_exp):
    """Build the 8 per-core input maps from the full-problem arrays."""
    x = np.ascontiguousarray(x, dtype=np.float32)
    w_gate = np.ascontiguousarray(w_gate, dtype=np.float32)
    w_exp = np.ascontiguousarray(w_exp, dtype=np.float32)
    b_exp = np.ascontiguousarray(b_exp, dtype=np.float32)

    xf = x.reshape(B, C * S)
    wexp_r = w_exp.reshape(E, 2, P, C)
    ident = np.eye(16, dtype=np.float32)

    in_maps = []
    for m in range(NCORES):
        ks = slice(m * KL, (m + 1) * KL)
        in_maps.append({
            "xg": np.ascontiguousarray(xf[:, ks].T),
            "wg": w_gate[ks],
            "xl": x[m * BL:(m + 1) * BL].reshape(BL, C, S),
            "wexp": wexp_r,
            "bexp": b_exp,
            "ident": ident,
        })
    return in_maps


_NC_CACHE = {}


def get_program(ffn_dt=FFN_DT):
    if ffn_dt not in _NC_CACHE:
        _NC_CACHE[ffn_dt] = build_program(ffn_dt)
    return _NC_CACHE[ffn_dt]


def run_sharded(inputs, trace=False, ffn_dt=FFN_DT):
    nc = get_program(ffn_dt)
    in_maps = shard_inputs(inputs["x"], inputs["w_gate"], inputs["w_exp"],
                           inputs["b_exp"])
    res = run_bass_kernel_spmd(nc, in_maps, list(range(NCORES)), trace=trace)
    y = np.concatenate([r["y"].reshape(BL, C, 32, 32) for r in res.results])
    gates = np.concatenate([r["gates"] for r in res.results])
    return (y, gates), res


def kernel(x, w_gate, w_exp, b_exp, k):
    assert int(k) == 4, f"kernel hardcodes top-4 gating, got k={k}"
    (y, gates), _ = run_sharded(
        {"x": np.asarray(x), "w_gate": np.asarray(w_gate),
         "w_exp": np.asarray(w_exp), "b_exp": np.asarray(b_exp)})
    return y, gates


# revision 11
# speedup vs baseline: 40686.6836x; 1.1968x over previous
"""Trainium2 Bass kernel for MoE routing (nn_MoE_mod_67224828117003).

Reference computation (per full batch):
  logits = x.reshape(B, C*H*W) @ w_gate            # [B, E]
  top-4 gating -> gk (softmax over top-4 logits), gates (dense scatter)
  out[b,j]     = w_exp[e_bj].T-contracted 1x1 conv of x[b] + b_exp[e_bj]
  y            = log(sum_j gk[b,j] * exp(out[b,j]))
  returns (y, gates)

Distribution over 8 NeuronCores:
  - Gating: K-split. Core m computes partial logits for ALL 128 samples over
    its 1/8 slice of the C*H*W contraction (host pre-transposes that slice so
    the contraction lands on SBUF partitions). One 8KB ReduceScatter then gives
    each core the final logits for ITS 16 samples.
  - Everything else is data-parallel over batch: each core does top-k, gathers
    its samples' 4 expert weight matrices from HBM via register-offset dynamic
    DMAs, runs the per-(sample, expert) matmuls on PE, and fuses
    gk*exp(out + b_exp) as exp(out + (b_exp + ln gk)) using the ACT engine's
    free per-partition bias.
"""
import sys

if '/opt/trn_rl_repo' not in sys.path:
    sys.path.insert(0, '/opt/trn_rl_repo')

import numpy as np

import concourse.bass as bass
import concourse.bacc as bacc
import concourse.mybir as mybir
from concourse import tile
from concourse.bass_utils import run_bass_kernel_spmd

F32 = mybir.dt.float32
U32 = mybir.dt.uint32
I32 = mybir.dt.int32
AF = mybir.ActivationFunctionType
ALU = mybir.AluOpType

B, C, S, E = 128, 256, 1024, 16          # batch, channels, H*W, experts
NCORES = 8
BL = B // NCORES                          # 16 local samples per core
KL = (C * S) // NCORES                    # 32768 gating-contraction slice
P = 128
KT = KL // P                              # 256 gating k-tiles
GCHUNK = 16                               # k-tiles per gating DMA batch

# FFN matmul dtype. float32r runs the PE at 1 col/cycle (vs 4 for float32) at
# reduced multiply precision (~tf32); gating stays float32 (expert selection
# needs full precision: min 4th/5th logit gap on this input is ~1.3e-3).
FFN_DT = mybir.dt.float32r


def _patch_act_tables(nc):
    """Force Exp and Ln to resolve to the one table set containing BOTH, so
    the ACT engine doesn't thrash table loads between interleaved exp/ln."""
    from concourse.hw_specs import get_activation_tables
    tabs = get_activation_tables(nc.m.arch)
    for name, fns in tabs.items():
        if name != "natural_log_exp_and_others":
            fns.discard(AF.Exp)
            fns.discard(AF.Ln)


def build_program(ffn_dt=FFN_DT):
    nc = bacc.Bacc("TRN2", target_bir_lowering=False, debug=False,
                   num_devices=NCORES)
    _patch_act_tables(nc)

    # host-preswizzled: xg[p, kt, s] = x.reshape(B,C*S)[s, m*KL + kt*128 + p]
    xg = nc.dram_tensor("xg", [P, KT, B], F32, kind="ExternalInput")
    # host-preswizzled: wg[p, kt*E + e] = w_gate[m*KL + kt*128 + p, e]
    wg = nc.dram_tensor("wg", [P, KT * E], F32, kind="ExternalInput")
    xl = nc.dram_tensor("xl", [BL, C, S], ffn_dt, kind="ExternalInput")
    wexp = nc.dram_tensor("wexp", [E, 2, P, C], ffn_dt, kind="ExternalInput")
    bexp = nc.dram_tensor("bexp", [E, C], F32, kind="ExternalInput")
    ident = nc.dram_tensor("ident", [16, 16], F32, kind="ExternalInput")

    y = nc.dram_tensor("y", [BL, 2, P, S], F32, kind="ExternalOutput")
    gates = nc.dram_tensor("gates", [BL, E], F32, kind="ExternalOutput")

    with tile.TileContext(nc) as tc:
        with tc.tile_pool(name="small", bufs=1) as small, \
             tc.tile_pool(name="dram", bufs=1, space="DRAM") as dr:

            # ---- Phase 1: gating partial logits over this core's K-slice ----
            lg_sb = small.tile([P, E], F32)
            with tc.tile_pool(name="gload", bufs=3) as gl, \
                 tc.tile_pool(name="gps", bufs=1, space="PSUM") as gps:
                lg_ps = gps.tile([P, E], F32)
                for g in range(KT // GCHUNK):
                    r0 = g * GCHUNK * P
                    xg_t = gl.tile([P, GCHUNK * B], F32, tag="xg")
                    nc.sync.dma_start(
                        xg_t[:],
                        xg[r0:r0 + GCHUNK * P, :].rearrange(
                            "(kt p) s -> p kt s", p=P))
                    wg_t = gl.tile([P, GCHUNK * E], F32, tag="wg")
                    nc.sync.dma_start(
                        wg_t[:],
                        wg[r0:r0 + GCHUNK * P, :].rearrange(
                            "(kt p) e -> p kt e", p=P))
                    for kt in range(GCHUNK):
                        nc.tensor.matmul(
                            lg_ps[:],
                            lhsT=xg_t[:, kt * B:(kt + 1) * B],
                            rhs=wg_t[:, kt * E:(kt + 1) * E],
                            start=(g == 0 and kt == 0),
                            stop=(g == KT // GCHUNK - 1 and kt == GCHUNK - 1))
                nc.vector.tensor_copy(lg_sb[:], lg_ps[:])

            # ---- Phase 2: ReduceScatter -> this core's final logits ----
            cc_in = dr.tile([P, E], F32)
            cc_out = dr.tile([BL, E], F32)
            nc.sync.dma_start(cc_in[:], lg_sb[:])
            nc.gpsimd.collective_compute(
                "ReduceScatter", ALU.add,
                replica_groups=[list(range(NCORES))],
                ins=[cc_in.opt()], outs=[cc_out.opt()])
            lg_loc = small.tile([BL, E], F32)
            nc.sync.dma_start(lg_loc[:], cc_out[:])

            # ---- Phase 3: top-4, softmax, gates, bias table ----
            m8 = small.tile([BL, 8], F32)
            i8 = small.tile([BL, 8], U32)
            nc.vector.max(out=m8[:], in_=lg_loc[:])
            nc.vector.max_index(out=i8[:], in_max=m8[:], in_values=lg_loc[:])

            d = small.tile([BL, 4], F32)
            nc.vector.tensor_scalar(d[:], m8[:, 0:4], m8[:, 0:1], None,
                                    op0=ALU.subtract)
            ed = small.tile([BL, 4], F32)
            nc.scalar.activation(ed[:], d[:], AF.Exp)
            ssum = small.tile([BL, 1], F32)
            nc.vector.tensor_reduce(ssum[:], ed[:], axis=mybir.AxisListType.X,
                                    op=ALU.add)
            rsum = small.tile([BL, 1], F32)
            nc.vector.reciprocal(rsum[:], ssum[:])
            gk = small.tile([BL, 4], F32)
            nc.vector.tensor_scalar_mul(gk[:], ed[:], rsum[:])
            lnsum = small.tile([BL, 1], F32)
            nc.scalar.activation(lnsum[:], ssum[:], AF.Ln)
            lngk = small.tile([BL, 4], F32)
            nc.vector.tensor_scalar(lngk[:], d[:], lnsum[:], None,
                                    op0=ALU.subtract)

            ident_sb = small.tile([16, 16], F32)
            nc.sync.dma_start(ident_sb[:], ident[:])
            gates_sb = small.tile([BL, E], F32)
            nc.vector.memset(gates_sb[:], 0.0)
            gt_sb = small.tile([16, 4 * BL], F32)
            bias_all = small.tile([P, 2 * 4 * BL], F32)
            with tc.tile_pool(name="tps", bufs=1, space="PSUM") as tps:
                for r in range(4):
                    eq = small.tile([BL, E], F32, tag=f"eq{r}")
                    nc.vector.tensor_scalar(eq[:], lg_loc[:], m8[:, r:r + 1],
                                            None, op0=ALU.is_equal)
                    nc.vector.scalar_tensor_tensor(
                        out=gates_sb[:], in0=eq[:], scalar=gk[:, r:r + 1],
                        in1=gates_sb[:], op0=ALU.mult, op1=ALU.add)
                    tp = tps.tile([16, 16], F32, tag=f"tp{r}")
                    nc.tensor.transpose(tp[:], eq[:], ident_sb[:])
                    nc.vector.tensor_copy(gt_sb[:, r * BL:(r + 1) * BL], tp[:])
                nc.sync.dma_start(gates[:], gates_sb[:])

                # bias_all[:, mc*64 + j*16 + s] = b_exp[e_sj, mc*128+p] + ln gk[s,j]
                lngk_d = dr.tile([1, 4 * BL], F32)
                nc.sync.dma_start(
                    lngk_d[0:1, :].rearrange("o (j s) -> o s j", s=BL),
                    lngk[:])
                lngk_bc = small.tile([P, 4 * BL], F32)
                nc.sync.dma_start(lngk_bc[:],
                                  lngk_d[0:1, :].to_broadcast([P, 4 * BL]))
                for mc in range(2):
                    bx = small.tile([16, P], F32, tag=f"bx{mc}")
                    nc.sync.dma_start(bx[:], bexp[:, mc * P:(mc + 1) * P])
                    bps = tps.tile([P, 4 * BL], F32, tag=f"bps{mc}")
                    nc.tensor.matmul(bps[:], lhsT=bx[:], rhs=gt_sb[:],
                                     start=True, stop=True)
                    nc.vector.tensor_add(
                        bias_all[:, mc * 4 * BL:(mc + 1) * 4 * BL],
                        bps[:], lngk_bc[:])

            # ---- Phase 4: per-sample expert FFN + combine ----
            with tc.tile_pool(name="ffn", bufs=2) as fp, \
                 tc.tile_pool(name="fwt", bufs=4) as fw, \
                 tc.tile_pool(name="fps", bufs=2, space="PSUM") as fps:
                for s in range(BL):
                    xt = []
                    for kc in range(2):
                        t = fp.tile([P, S], ffn_dt, tag=f"xt{kc}")
                        nc.sync.dma_start(t[:], xl[s, kc * P:(kc + 1) * P, :])
                        xt.append(t)
                    acc = fp.tile([P, 2 * S], F32, tag="acc")
                    for j in range(4):
                        ev = nc.values_load(
                            i8[s:s + 1, j:j + 1].bitcast(I32),
                            engines=(mybir.EngineType.SP,),
                            min_val=0, max_val=E - 1,
                            skip_runtime_bounds_check=True)
                        wt = fw.tile([P, 2 * C], ffn_dt, tag="wt")
                        nc.sync.dma_start(
                            wt[:],
                            wexp[bass.ds(ev, 1), :, :, :].rearrange(
                                "e kc p o -> e p kc o"))
                        ps0 = fps.tile([P, S], F32, tag="ps0")
                        ps1 = fps.tile([P, S], F32, tag="ps1")
                        ps = [ps0, ps1]
                        for mc in range(2):
                            for kc in range(2):
                                lhsT = wt[:, kc * C + mc * P:kc * C + mc * P + P]
                                for n_ in range(2):
                                    nc.tensor.matmul(
                                        ps[mc][:, n_ * 512:(n_ + 1) * 512],
                                        lhsT=lhsT,
                                        rhs=xt[kc][:, n_ * 512:(n_ + 1) * 512],
                                        start=(kc == 0), stop=(kc == 1))
                        for mc in range(2):
                            bias_ap = bias_all[:, mc * 4 * BL + j * BL + s:
                                               mc * 4 * BL + j * BL + s + 1]
                            if j == 0:
                                nc.scalar.activation(
                                    acc[:, mc * S:(mc + 1) * S], ps[mc][:],
                                    AF.Exp, bias=bias_ap)
                            else:
                                et = fp.tile([P, S], F32, tag="etmp")
                                nc.scalar.activation(et[:], ps[mc][:],
                                                     AF.Exp, bias=bias_ap)
                                nc.vector.tensor_add(
                                    acc[:, mc * S:(mc + 1) * S],
                                    acc[:, mc * S:(mc + 1) * S], et[:])
                    yt = fp.tile([P, 2 * S], F32, tag="yt")
                    nc.scalar.activation(yt[:], acc[:], AF.Ln)
                    nc.sync.dma_start(
                        y[s].rearrange("mc p hw -> p mc hw"), yt[:])

    nc.compile()
    return nc


def shard_inputs(x, w_gate, w_exp, b_exp):
    """Build the 8 per-core input maps from the full-problem arrays."""
    x = np.ascontiguousarray(x, dtype=np.float32)
    w_gate = np.ascontiguousarray(w_gate, dtype=np.float32)
    w_exp = np.ascontiguousarray(w_exp, dtype=np.float32)
    b_exp = np.ascontiguousarray(b_exp, dtype=np.float32)

    xf = x.reshape(B, C * S)
    wexp_r = w_exp.reshape(E, 2, P, C)
    ident = np.eye(16, dtype=np.float32)

    in_maps = []
    for m in range(NCORES):
        ks = slice(m * KL, (m + 1) * KL)
        in_maps.append({
            "xg": np.ascontiguousarray(xf[:, ks].T),
            "wg": w_gate[ks],
            "xl": x[m * BL:(m + 1) * BL].reshape(BL, C, S),
            "wexp": wexp_r,
            "bexp": b_exp,
            "ident": ident,
        })
    return in_maps


_NC_CACHE = {}


def get_program(ffn_dt=FFN_DT):
    if ffn_dt not in _NC_CACHE:
        _NC_CACHE[ffn_dt] = build_program(ffn_dt)
    return _NC_CACHE[ffn_dt]


def run_sharded(inputs, trace=False, ffn_dt=FFN_DT):
    nc = get_program(ffn_dt)
    in_maps = shard_inputs(inputs["x"], inputs["w_gate"], inputs["w_exp"],
                           inputs["b_exp"])
    res = run_bass_kernel_spmd(nc, in_maps, list(range(NCORES)), trace=trace)
    y = np.concatenate([r["y"].reshape(BL, C, 32, 32) for r in res.results])
    gates = np.concatenate([r["gates"] for r in res.results])
    return (y, gates), res


def kernel(x, w_gate, w_exp, b_exp, k):
    assert int(k) == 4, f"kernel hardcodes top-4 gating, got k={k}"
    (y, gates), _ = run_sharded(
        {"x": np.asarray(x), "w_gate": np.asarray(w_gate),
         "w_exp": np.asarray(w_exp), "b_exp": np.asarray(b_exp)})
    return y, gates
